# revision 20
# baseline (speedup 1.0000x reference)
"""Trainium2 Bass kernel for nn_DS4DKernel_56504589746318.

Math (per batch b):
    deltaA = W @ du[b]              # (N=64, L=4096)
    S      = cumsum_L(deltaA)       # (64, 4096)  -- tensor_tensor_scan
    K[b]   = (C*Bvec) @ S + base    # (H=1024, L=4096), base = C @ (A @ Bvec)

Sharding: data-parallel over batch, one batch per NeuronCore (B=8 = 8 cores).
Small matrices (W^T, (C*Bvec)^T, base) are precomputed on host and replicated.

I/O runs in bf16 (du cast on host, K cast back after) which halves HBM
traffic; the 2e-2 tolerance leaves ample room (bf16 lands ~3e-3).
PE work is fixed at 65536 moving rows (mm1 8 contraction chunks x 4096 +
mm2 8 output chunks x 4096), which at the throttled ~1.3 GHz clock is the
co-bottleneck with DMA, so the schedule aims to keep PE busy from the
first kilobyte: wt lands first via HWDGE on Sync, du streams in 512-col
tiles, outputs drain through gpsimd SWDGE so input dispatch never queues
behind output dispatch.
"""

import sys

for _p in ("/opt/trn_rl_repo", "/root/.axon_site/_ro/trn_rl_repo"):
    if _p not in sys.path:
        sys.path.insert(0, _p)

import ml_dtypes
import numpy as np

import concourse.bass as bass
import concourse.mybir as mybir
import concourse.tile as tile
from concourse import bacc
from concourse.bass_utils import run_bass_kernel_spmd

B, H, N, L = 8, 1024, 64, 4096
P = 128          # SBUF partitions
HC = H // P      # 8 h-chunks of 128
LT = 512         # l-tile width = one PSUM bank of f32, one matmul moving dim
NLT = L // LT    # 8 l-tiles
N1 = N + 1       # deltaA/S/ccbt carry an extra all-ones/base row so mm2
                 # adds base for free (wt column 64 is zero-padded on host)

F32 = mybir.dt.float32
F32R = mybir.dt.float32r
BF16 = mybir.dt.bfloat16
ADD = mybir.AluOpType.add
BYPASS = mybir.AluOpType.bypass


def build_nc():
    nc = bacc.Bacc()
    du_d = nc.declare_dram_parameter("du", [H, L], BF16, isOutput=False)
    wt_d = nc.declare_dram_parameter("wt", [H, N1], BF16, isOutput=False)
    ccbt_d = nc.declare_dram_parameter("ccbt", [N1, H], BF16, isOutput=False)
    out_d = nc.declare_dram_parameter("out", [H, L], BF16, isOutput=True)

    with tile.TileContext(nc) as tc:
        with (
            tc.tile_pool(name="const", bufs=1) as cpool,
            tc.tile_pool(name="du", bufs=3) as dupool,
            tc.tile_pool(name="s", bufs=2) as spool,
            tc.tile_pool(name="outp", bufs=3) as opool,
            tc.tile_pool(name="psA", bufs=2, space="PSUM") as psA,
            tc.tile_pool(name="psB", bufs=3, space="PSUM") as psB,
        ):
            du_t = [None] * NLT
            dA_t = [None] * NLT
            S_t = [None] * NLT

            # --- constants, part 1: wt must land before the first matmul,
            # so it goes out first on the Sync HWDGE queue (fast spin-up).
            wt_sb = cpool.tile([P, HC, N1], BF16)    # [p, c, n] = W^T[c*128+p, n]
            nc.sync.dma_start(
                wt_sb[:], wt_d[:, :].rearrange("(c p) n -> p c n", p=P)
            )

            def load_du(lt, split=2):
                # HWDGE via Sync, split so downstream matmuls start after
                # the first piece lands
                du_t[lt] = dupool.tile([P, HC, LT], BF16, tag="du_t", name="du_t")
                for g in range(split):
                    c0, c1 = g * HC // split, (g + 1) * HC // split
                    nc.sync.dma_start(
                        du_t[lt][:, c0:c1, :],
                        du_d[
                            c0 * P : c1 * P, lt * LT : (lt + 1) * LT
                        ].rearrange("(c p) j -> p c j", p=P),
                    )

            load_du(0, split=4)

            # --- constants, part 2 (small / non-urgent, off the Sync queue)
            ccbt_sb = cpool.tile([N1, H], BF16)      # [n, h] = (C*Bvec)^T; base
            nc.gpsimd.dma_start(ccbt_sb[:], ccbt_d[:, :])
            zeros_sb = cpool.tile([N1, LT], F32)     # data1 for the scan
            nc.vector.memset(zeros_sb[:], 0.0)
            init_sb = cpool.tile([N1, 1], F32)       # scan seed: 0s, ones row 1
            nc.vector.memset(init_sb[:], 0.0)
            nc.vector.memset(init_sb[N:N1, :], 1.0)

            load_du(1)

            def mm1(lt):
                # deltaA tile: accumulate over 8 h-chunks into PSUM.  Row 64
                # is written too (wt column 64 is zero) so it's exactly 0.
                dA_t[lt] = psA.tile([N1, LT], F32, tag="dA_t", name="dA_t")
                for c in range(HC):
                    nc.tensor.matmul(
                        dA_t[lt][:],
                        wt_sb[:, c, :],
                        du_t[lt][:, c, :],
                        start=(c == 0),
                        stop=(c == HC - 1),
                    )

            def scan(lt):
                # scan state is fp32 internally regardless of output dtype,
                # so a bf16 S costs one rounding per element (plus one per
                # tile-boundary carry), well inside the 2e-2 budget.  Row 64
                # scans 0s from a seed of 1.0, i.e. stays exactly 1.0 -- the
                # ones row that makes mm2 add base.
                S_t[lt] = spool.tile([N1, LT], BF16, tag="S_t", name="S_t")
                initial = init_sb[:] if lt == 0 else S_t[lt - 1][:, LT - 1 : LT]
                nc.vector.tensor_tensor_scan(
                    S_t[lt][:], dA_t[lt][:], zeros_sb[:], initial,
                    op0=ADD, op1=BYPASS,
                )

            def mm2_and_out(lt, out_split=2, act_frac=2):
                last = lt == NLT - 1
                out_sb = opool.tile([P, HC, LT], BF16)
                cg = HC // out_split  # h-chunks per out-DMA
                for cp in range(HC // 2):
                    # two h-chunks share a 2-bank PSUM tile so the
                    # PSUM->SBUF copy runs once per pair
                    po = psB.tile([P, 2, LT], F32, tag="po", name="po")
                    for ci in range(2):
                        c = 2 * cp + ci
                        nc.tensor.matmul(
                            po[:, ci, :],
                            ccbt_sb[:, c * P : (c + 1) * P],
                            S_t[lt][:],
                            start=True,
                            stop=True,
                        )
                    # PSUM -> SBUF (bf16); copies split between DVE and ACT
                    dst = out_sb[:, 2 * cp : 2 * cp + 2, :]
                    if cp % act_frac == act_frac - 1:
                        nc.scalar.copy(dst, po[:])
                    else:
                        nc.vector.tensor_scalar_add(dst, po[:], 0.0)
                    c = 2 * cp + 1
                    if (c + 1) % cg == 0:
                        g0 = c + 1 - cg
                        # steady-state outs ride the idle gpsimd SWDGE
                        # queue; the final tile drains via Sync HWDGE for
                        # the shortest tail
                        eng = nc.sync if last else nc.gpsimd
                        eng.dma_start(
                            out_d[
                                g0 * P : (c + 1) * P, lt * LT : (lt + 1) * LT
                            ].rearrange("(c p) j -> p c j", p=P),
                            out_sb[:, g0 : c + 1, :],
                        )

            # software-pipelined emission.  mm2(lt-1) is emitted BEFORE
            # mm1(lt) so PE always has ready work queued (PE executes in
            # program order).
            mm1(0)
            scan(0)
            for lt in range(1, NLT):
                if lt + 1 < NLT:
                    load_du(lt + 1)
                mm2_and_out(lt - 1)
                mm1(lt)
                scan(lt)
            mm2_and_out(NLT - 1, out_split=4, act_frac=2)

    nc.compile()
    return nc


_NC_CACHE = None


def _get_nc():
    global _NC_CACHE
    if _NC_CACHE is None:
        _NC_CACHE = build_nc()
    return _NC_CACHE


def _prep_in_maps(du, C, Bvec, A, W):
    du = np.asarray(du, dtype=np.float32).astype(ml_dtypes.bfloat16)
    du = np.ascontiguousarray(du)
    C = np.asarray(C, dtype=np.float32)
    Bvec = np.asarray(Bvec, dtype=np.float32)
    A = np.asarray(A, dtype=np.float32)
    W = np.asarray(W, dtype=np.float32)

    # wt gets a zero 65th column (keeps deltaA row 64 at exactly 0); ccbt
    # gets base as a 65th row (mm2's ones row in S turns it into "+ base")
    wt = np.zeros((H, N1), dtype=ml_dtypes.bfloat16)
    wt[:, :N] = W.T.astype(ml_dtypes.bfloat16)
    base = C @ (A @ Bvec)                               # (H,)
    ccbt = np.empty((N1, H), dtype=ml_dtypes.bfloat16)
    ccbt[:N] = (C * Bvec[None, :]).T.astype(ml_dtypes.bfloat16)
    ccbt[N] = base.astype(ml_dtypes.bfloat16)

    return [{"du": du[b], "wt": wt, "ccbt": ccbt} for b in range(B)]


def run(du, C, Bvec, A, W, trace=False):
    nc = _get_nc()
    in_maps = _prep_in_maps(du, C, Bvec, A, W)
    res = run_bass_kernel_spmd(nc, in_maps, core_ids=list(range(B)), trace=trace)
    out = np.stack(
        [res.results[b]["out"].astype(np.float32) for b in range(B)], axis=0
    )
    return out, res


def kernel(du, C, Bvec, A, W):
    out, _ = run(du, C, Bvec, A, W, trace=False)
    return out


# revision 43
# speedup vs baseline: 1.0747x; 1.0747x over previous
"""Trainium2 Bass kernel for nn_DS4DKernel_56504589746318.

Math (per batch b):
    deltaA = W @ du[b]              # (N=64, L=4096)
    S      = cumsum_L(deltaA)       # (64, 4096)  -- tensor_tensor_scan
    K[b]   = (C*Bvec) @ S + base    # (H=1024, L=4096), base = C @ (A @ Bvec)

Sharding: data-parallel over batch, one batch per NeuronCore (B=8 = 8 cores).
Small matrices (W^T, (C*Bvec)^T, base) are precomputed on host and replicated.

I/O runs in bf16 (du cast on host, K cast back after) which halves HBM
traffic; the 2e-2 tolerance leaves ample room (bf16 lands ~3e-3).
PE work is fixed at 65536 moving rows (mm1 8 contraction chunks x 4096 +
mm2 8 output chunks x 4096), which at the throttled ~1.3 GHz clock is the
co-bottleneck with DMA, so the schedule aims to keep PE busy from the
first kilobyte: wt lands first via HWDGE on Sync, du streams in 512-col
tiles, outputs drain through gpsimd SWDGE so input dispatch never queues
behind output dispatch.
"""

import sys

for _p in ("/opt/trn_rl_repo", "/root/.axon_site/_ro/trn_rl_repo"):
    if _p not in sys.path:
        sys.path.insert(0, _p)

import ml_dtypes
import numpy as np

import concourse.bass as bass
import concourse.mybir as mybir
import concourse.tile as tile
from concourse import bacc
from concourse.bass_utils import run_bass_kernel_spmd

B, H, N, L = 8, 1024, 64, 4096
P = 128          # SBUF partitions
HC = H // P      # 8 h-chunks of 128
LT = 512         # l-tile width = one PSUM bank of f32, one matmul moving dim
NLT = L // LT    # 8 l-tiles
N1 = N + 1       # deltaA/S/ccbt carry an extra all-ones/base row so mm2
                 # adds base for free (wt column 64 is zero-padded on host)

F32 = mybir.dt.float32
F32R = mybir.dt.float32r
BF16 = mybir.dt.bfloat16
ADD = mybir.AluOpType.add
BYPASS = mybir.AluOpType.bypass


def build_nc():
    nc = bacc.Bacc()
    du_d = nc.declare_dram_parameter("du", [H, L], BF16, isOutput=False)
    # wt arrives pre-swizzled to the SBUF layout: one contiguous 1040B line
    # per partition instead of 1024 130B gather lines
    wt_d = nc.declare_dram_parameter("wt", [P, HC * N1], BF16, isOutput=False)
    ccbt_d = nc.declare_dram_parameter("ccbt", [N1, H], BF16, isOutput=False)
    out_d = nc.declare_dram_parameter("out", [H, L], BF16, isOutput=True)

    with tile.TileContext(nc) as tc:
        with (
            tc.tile_pool(name="const", bufs=1) as cpool,
            tc.tile_pool(name="du", bufs=3) as dupool,
            tc.tile_pool(name="s", bufs=2) as spool,
            tc.tile_pool(name="outp", bufs=3) as opool,
            tc.tile_pool(name="psA", bufs=2, space="PSUM") as psA,
            tc.tile_pool(name="psB", bufs=3, space="PSUM") as psB,
        ):
            du_p = [None] * (NLT // 2)
            dA_t = [None] * NLT
            S_t = [None] * NLT

            # --- constants, part 1: wt must land before the first matmul,
            # so it goes out first on the Sync HWDGE queue (fast spin-up).
            wt_sb = cpool.tile([P, HC, N1], BF16)    # [p, c, n] = W^T[c*128+p, n]
            nc.sync.dma_start(
                wt_sb[:], wt_d[:, :].rearrange("p (c n) -> p c n", c=HC)
            )

            def load_pair(pr, fine=False):
                # du streams as PAIRS of l-tiles on the Sync HWDGE queue so
                # each DRAM line is 2 KB (HWDGE doesn't aggregate packets);
                # the first pair loads in l-tile-sized pieces instead so
                # mm1(0) starts as early as possible
                du_p[pr] = dupool.tile(
                    [P, HC, 2, LT], BF16, tag="du_p", name="du_p"
                )
                if fine:
                    # first tile's pieces split across the ACT and Sync
                    # HWDGE rings so dispatch doesn't fully serialize; the
                    # ACT ring is idle at startup and unused after
                    engs = [nc.scalar, nc.sync, nc.scalar, nc.sync]
                    for lh in range(2):
                        for g in range(2):
                            c0, c1 = g * HC // 2, (g + 1) * HC // 2
                            l0 = pr * 2 * LT + lh * LT
                            engs[lh * 2 + g].dma_start(
                                du_p[pr][:, c0:c1, lh, :],
                                du_d[
                                    c0 * P : c1 * P, l0 : l0 + LT
                                ].rearrange("(c p) j -> p c j", p=P),
                            )
                else:
                    # pairs 1-2 also use the otherwise-idle ACT ring for
                    # their first half; later pairs keep Sync (ACT is busy
                    # with copies by then)
                    engs = [nc.scalar, nc.sync] if pr <= 2 else [nc.sync] * 2
                    for g in range(2):
                        c0, c1 = g * HC // 2, (g + 1) * HC // 2
                        engs[g].dma_start(
                            du_p[pr][:, c0:c1, :, :],
                            du_d[
                                c0 * P : c1 * P,
                                pr * 2 * LT : (pr + 1) * 2 * LT,
                            ].rearrange("(c p) (h j) -> p c h j", p=P, h=2),
                        )

            load_pair(0, fine=True)

            # --- constants, part 2 (small / non-urgent, off the Sync queue)
            ccbt_sb = cpool.tile([N1, H], BF16)      # [n, h] = (C*Bvec)^T; base
            nc.gpsimd.dma_start(ccbt_sb[:], ccbt_d[:, :])
            zeros_sb = cpool.tile([N1, LT], F32)     # data1 for the scan
            nc.vector.memset(zeros_sb[:], 0.0)
            init_sb = cpool.tile([N1, 1], F32)       # scan seed: 0s, ones row 1
            nc.vector.memset(init_sb[:], 0.0)
            nc.vector.memset(init_sb[N:N1, :], 1.0)

            load_pair(1)

            # PE warm-up: garbage bf16 matmuls ramp the tensor engine's
            # p-state while the first du pieces are still in flight, so the
            # first real matmuls run at speed.  Output goes to the psA
            # buffer that dA(1) will overwrite anyway.
            warm_sb = cpool.tile([P, 256], BF16)
            nc.vector.memset(warm_sb[:], 0.0)
            warm_ps = psA.tile([N1, LT], F32, tag="dA_t", name="dA_t")
            for _ in range(10):
                nc.tensor.matmul(
                    warm_ps[0:N, 0:256],
                    warm_sb[:, 0:N],
                    warm_sb[:],
                    start=True,
                    stop=True,
                )

            # third pair starts streaming before compute begins; dupool has
            # three buffers, so the input stream runs a full pair ahead of
            # the consumer and rides out mid-kernel DMA contention
            load_pair(2)

            def mm1(lt):
                # deltaA tile: accumulate over 8 h-chunks into PSUM.  Row 64
                # is written too (wt column 64 is zero) so it's exactly 0.
                dA_t[lt] = psA.tile([N1, LT], F32, tag="dA_t", name="dA_t")
                for c in range(HC):
                    nc.tensor.matmul(
                        dA_t[lt][:],
                        wt_sb[:, c, :],
                        du_p[lt // 2][:, c, lt % 2, :],
                        start=(c == 0),
                        stop=(c == HC - 1),
                    )

            def scan(lt):
                # scan state is fp32 internally regardless of output dtype,
                # so a bf16 S costs one rounding per element (plus one per
                # tile-boundary carry), well inside the 2e-2 budget.  Row 64
                # scans 0s from a seed of 1.0, i.e. stays exactly 1.0 -- the
                # ones row that makes mm2 add base.
                S_t[lt] = spool.tile([N1, LT], BF16, tag="S_t", name="S_t")
                initial = init_sb[:] if lt == 0 else S_t[lt - 1][:, LT - 1 : LT]
                nc.vector.tensor_tensor_scan(
                    S_t[lt][:], dA_t[lt][:], zeros_sb[:], initial,
                    op0=ADD, op1=BYPASS,
                )

            def mm2_and_out(lt, out_split=2, act_frac=2):
                last = lt == NLT - 1
                out_sb = opool.tile([P, HC, LT], BF16)
                cg = HC // out_split  # h-chunks per out-DMA
                for cp in range(HC // 2):
                    # two h-chunks share a 2-bank PSUM tile so the
                    # PSUM->SBUF copy runs once per pair (the final tile
                    # copies per-chunk on both engines to drain faster)
                    po = psB.tile([P, 2, LT], F32, tag="po", name="po")
                    for ci in range(2):
                        c = 2 * cp + ci
                        nc.tensor.matmul(
                            po[:, ci, :],
                            ccbt_sb[:, c * P : (c + 1) * P],
                            S_t[lt][:],
                            start=True,
                            stop=True,
                        )
                        if last:
                            dst = out_sb[:, c, :]
                            if ci == 0:
                                nc.vector.tensor_scalar_add(
                                    dst, po[:, ci, :], 0.0
                                )
                            else:
                                nc.scalar.copy(dst, po[:, ci, :])
                    if not last:
                        # PSUM -> SBUF (bf16); DVE takes only one pair per
                        # tile so the scan (which gates the next mm2) never
                        # queues behind copies; ACT takes the rest
                        dst = out_sb[:, 2 * cp : 2 * cp + 2, :]
                        if cp == 0:
                            nc.vector.tensor_scalar_add(dst, po[:], 0.0)
                        else:
                            nc.scalar.copy(dst, po[:])
                    c = 2 * cp + 1
                    if (c + 1) % cg == 0:
                        g0 = c + 1 - cg
                        # steady-state outs ride the idle gpsimd SWDGE
                        # queue; the final tile drains via Sync HWDGE for
                        # the shortest tail
                        eng = nc.sync if last else nc.gpsimd
                        eng.dma_start(
                            out_d[
                                g0 * P : (c + 1) * P, lt * LT : (lt + 1) * LT
                            ].rearrange("(c p) j -> p c j", p=P),
                            out_sb[:, g0 : c + 1, :],
                        )

            # software-pipelined emission.  mm2(lt-1) is emitted BEFORE
            # mm1(lt) so PE always has ready work queued (PE executes in
            # program order).
            mm1(0)
            scan(0)
            for lt in range(1, NLT):
                if lt == 2:
                    load_pair(3)
                mm2_and_out(lt - 1)
                mm1(lt)
                scan(lt)
            mm2_and_out(NLT - 1, out_split=4, act_frac=2)

    nc.compile()
    return nc


_NC_CACHE = None


def _get_nc():
    global _NC_CACHE
    if _NC_CACHE is None:
        _NC_CACHE = build_nc()
    return _NC_CACHE


def prep_in_maps(du, C, Bvec, A, W):
    du = np.asarray(du, dtype=np.float32).astype(ml_dtypes.bfloat16)
    du = np.ascontiguousarray(du)
    C = np.asarray(C, dtype=np.float32)
    Bvec = np.asarray(Bvec, dtype=np.float32)
    A = np.asarray(A, dtype=np.float32)
    W = np.asarray(W, dtype=np.float32)

    # wt gets a zero 65th column (keeps deltaA row 64 at exactly 0); ccbt
    # gets base as a 65th row (mm2's ones row in S turns it into "+ base").
    # wt is pre-swizzled to the on-chip [p, c, n] layout.
    wt = np.zeros((H, N1), dtype=ml_dtypes.bfloat16)
    wt[:, :N] = W.T.astype(ml_dtypes.bfloat16)
    wt = np.ascontiguousarray(
        wt.reshape(HC, P, N1).transpose(1, 0, 2).reshape(P, HC * N1)
    )
    base = C @ (A @ Bvec)                               # (H,)
    ccbt = np.empty((N1, H), dtype=ml_dtypes.bfloat16)
    ccbt[:N] = (C * Bvec[None, :]).T.astype(ml_dtypes.bfloat16)
    ccbt[N] = base.astype(ml_dtypes.bfloat16)

    return [{"du": du[b], "wt": wt, "ccbt": ccbt} for b in range(B)]


def run(du, C, Bvec, A, W, trace=False):
    nc = _get_nc()
    in_maps = prep_in_maps(du, C, Bvec, A, W)
    res = run_bass_kernel_spmd(nc, in_maps, core_ids=list(range(B)), trace=trace)
    out = np.stack(
        [res.results[b]["out"].astype(np.float32) for b in range(B)], axis=0
    )
    return out, res


def kernel(du, C, Bvec, A, W):
    out, _ = run(du, C, Bvec, A, W, trace=False)
    return out
